# revision 42
# baseline (speedup 1.0000x reference)
"""Block sliding-window attention on 8 TRN2 NeuronCores.

Sharding: sequence-parallel. 8 shards = (batch b in {0,1}) x (quarter s in
0..3); each core owns 2048 consecutive tokens of one batch plus a 256-token
K/V halo from the previous quarter (zeros + -inf gate for the first quarter).
No collectives: each core computes its tokens' full output rows.

v2 pipeline (vs v1: fp8 DoubleRow projections, RoPE fused into P1,
P2/P3 fused per chunk, no OTS round-trip):
  P1: Q^T/K^T/V via 3-term fp8 e4m3 hi/lo matmuls in DoubleRow perf mode
      (h = h8 + hlo at scale 8, W = W8 + Wlo at scale 256; terms
      h8*W8 + h8*Wlo + hlo*W8 share one PSUM at scale 2048; the dropped
      hlo*Wlo term is ~1e-3 relative). RoPE is applied to Q/K right after
      the PSUM drain (rot-half via SBUF->SBUF partition-swap DMA, cos/sin
      resident bf16), roped heads stored to DRAM scratch. V drains to DRAM
      natural-layout scratch.
  P2+P3 fused per 256-token chunk: scores S^T = K Q^T per 128-key block
      (skipping the fully-masked kb3 x first-half-queries block), exp on
      ACT (-1e30 bias gates the no-previous case), 0/1 triangular mask on
      DVE, denominator via all-ones matmul, O^T = V^T P^T, normalize
      (reciprocal on DVE, multiply on Pool engine), then immediately
      out += O_h @ Wo_h accumulated over 16 heads in PSUM - O^T never
      leaves SBUF.
"""
import sys

try:
    import concourse  # noqa: F401
except ImportError:
    sys.path.insert(0, '/opt/trn_rl_repo')

import ml_dtypes
import numpy as np

import concourse.bacc as bacc
import concourse.mybir as mybir
import concourse.tile as tile
from concourse.bass_utils import run_bass_kernel_spmd

f32 = mybir.dt.float32
f32r = mybir.dt.float32r
bf16 = mybir.dt.bfloat16
f8 = mybir.dt.float8e4
AF = mybir.ActivationFunctionType
DR = mybir.MatmulPerfMode.DoubleRow

DIMS = 2048
HEADS = 16
HD = 128           # head dim
WIN = 256          # window / chunk
B, S = 2, 8192
NSH = 4            # seq shards per batch
THETA = 10000.0
ISQ = float(1.0 / np.sqrt(HD))
KP = DIMS // 256   # 8 contraction k-pairs (256 rows each) for DoubleRow
SH_H = 8.0         # fp8 scale for hidden
SH_W = 256.0       # fp8 scale for weights
DESC = 1.0 / (SH_H * SH_W)


def tok_tiles(n, w=512):
    out, a = [], 0
    while a < n:
        out.append((a, min(w, n - a)))
        a += w
    return out


def build(nc, T, phases=(1, 2)):
    """Emit the per-core program. T = local tokens (multiple of 512)."""
    TH = T + WIN                      # with halo
    NC_ = T // WIN                    # chunks
    H8 = nc.dram_tensor("H8", [DIMS, TH], f8, kind="ExternalInput")
    HLO = nc.dram_tensor("HLO", [DIMS, TH], f8, kind="ExternalInput")
    WT8 = {}
    WTLO = {}
    for w_ in ("Q", "K", "V"):
        WT8[w_] = nc.dram_tensor(f"W{w_}8", [KP * 128, 2, DIMS], f8,
                                 kind="ExternalInput")
        WTLO[w_] = nc.dram_tensor(f"W{w_}LO", [KP * 128, 2, DIMS], f8,
                                  kind="ExternalInput")
    WO8 = nc.dram_tensor("WO8", [HEADS * 128, DIMS], f8, kind="ExternalInput")
    WOLO = nc.dram_tensor("WOLO", [HEADS * 128, DIMS], f8, kind="ExternalInput")
    COS = nc.dram_tensor("COS", [HD, TH], bf16, kind="ExternalInput")
    SINS = nc.dram_tensor("SINS", [HD, TH], bf16, kind="ExternalInput")
    TRI = nc.dram_tensor("TRI", [128, 384], bf16, kind="ExternalInput")
    PGATE = nc.dram_tensor("PGATE", [128, 1], f32, kind="ExternalInput")
    ONESM = nc.dram_tensor("ONESM", [128, 128], bf16, kind="ExternalInput")
    OUT = nc.dram_tensor("OUT", [T, DIMS], bf16, kind="ExternalOutput")

    KTS = nc.dram_tensor("KTS", [HEADS, HD, TH], bf16)   # roped K^T
    QTS = nc.dram_tensor("QTS", [HEADS, HD, T], bf16)    # roped Q^T
    VS = nc.dram_tensor("VS", [TH, DIMS], bf16)          # V natural

    with tile.TileContext(nc) as tc:
        # kq is opened at the outermost scope: its space never overlaps the
        # P1 pools, so P23 chunk-input prefetches have no buffer-reuse (WAR)
        # dependency on P1's tail and can land as soon as KTS/QTS/VS rows are
        # written.
        with tc.tile_pool(name="cst", bufs=1) as cst, \
             tc.tile_pool(name="kq", bufs=2) as kq:
            tri = cst.tile([128, 384], bf16)
            pgate = cst.tile([128, 1], f32)
            onesm = cst.tile([128, 128], bf16)
            nc.sync.dma_start(tri[:], TRI[:])
            nc.sync.dma_start(pgate[:], PGATE[:])
            nc.sync.dma_start(onesm[:], ONESM[:])

            # ---------------- P1: projections + RoPE ----------------
            if 1 in phases:
              with tc.tile_pool(name="hp", bufs=1) as hp, \
                 tc.tile_pool(name="rope_c", bufs=1) as rcp, \
                 tc.tile_pool(name="wp", bufs=1) as wp, \
                 tc.tile_pool(name="dr", bufs=2) as dr, \
                 tc.tile_pool(name="vst", bufs=2) as vst, \
                 tc.tile_pool(name="pp", bufs=8, space="PSUM") as pp:
                def load_w_half(which, half):
                    """Load output-column half of a weight pair."""
                    w8, wlo = [], []
                    csl = slice(half * DIMS // 2, (half + 1) * DIMS // 2)
                    for kp in range(KP):
                        a = wp.tile([128, 2, DIMS // 2], f8, name=f"w8_{kp}")
                        b_ = wp.tile([128, 2, DIMS // 2], f8, name=f"wlo_{kp}")
                        nc.sync.dma_start(a[:], WT8[which]
                                          .rearrange("(kp p) j c -> kp p j c",
                                                     p=128)[kp][:, :, csl])
                        nc.sync.dma_start(b_[:], WTLO[which]
                                          .rearrange("(kp p) j c -> kp p j c",
                                                     p=128)[kp][:, :, csl])
                        w8.append(a)
                        wlo.append(b_)
                    return w8, wlo

                # startup: interleave K-weight half 0 with the hidden loads
                # kp-wise so the kp0 matmuls can begin after ~2.4MB of DMA
                wk0_8, wk0_lo = [], []
                h8t, hlot = [], []
                for kp in range(KP):
                    wa = wp.tile([128, 2, DIMS // 2], f8, name=f"w8_{kp}")
                    wb = wp.tile([128, 2, DIMS // 2], f8, name=f"wlo_{kp}")
                    csl = slice(0, DIMS // 2)
                    nc.sync.dma_start(wa[:], WT8["K"]
                                      .rearrange("(kp p) j c -> kp p j c",
                                                 p=128)[kp][:, :, csl])
                    nc.sync.dma_start(wb[:], WTLO["K"]
                                      .rearrange("(kp p) j c -> kp p j c",
                                                 p=128)[kp][:, :, csl])
                    wk0_8.append(wa)
                    wk0_lo.append(wb)
                    a = hp.tile([128, 2, TH], f8, name=f"h8_{kp}")
                    b_ = hp.tile([128, 2, TH], f8, name=f"hlo_{kp}")
                    sl = slice(kp * 256, (kp + 1) * 256)
                    nc.sync.dma_start(
                        a[:], H8[sl].rearrange("(j p) t -> p j t", p=128))
                    nc.sync.dma_start(
                        b_[:], HLO[sl].rearrange("(j p) t -> p j t", p=128))
                    h8t.append(a)
                    hlot.append(b_)
                wk_halves = [(wk0_8, wk0_lo)]
                cosb = rcp.tile([128, TH], bf16)
                sinb = rcp.tile([128, TH], bf16)
                nc.sync.dma_start(cosb[:], COS[:])
                nc.sync.dma_start(sinb[:], SINS[:])

                def mm3(ps, lhs, rhs):
                    """3-term fp8 DoubleRow accumulate into one psum."""
                    n = len(lhs)
                    for kp in range(n):
                        (l8, llo), (r8, rlo) = lhs[kp], rhs[kp]
                        nc.tensor.matmul(ps, l8, r8, start=(kp == 0),
                                         stop=False, perf_mode=DR)
                        nc.tensor.matmul(ps, l8, rlo, start=False, stop=False,
                                         perf_mode=DR)
                        nc.tensor.matmul(ps, llo, r8, start=False,
                                         stop=(kp == n - 1), perf_mode=DR)

                # K then Q (head-transposed layout + RoPE), V natural last.
                # Weights stream in output-column halves.
                w_seq = [("K", 1), ("Q", 0), ("Q", 1), ("V", 0), ("V", 1)]
                for which, DST, t0, tlen in (("K", KTS, 0, TH),
                                             ("Q", QTS, WIN, T)):
                    for half in range(2):
                        w8, wlo = wk_halves.pop(0)
                        if w_seq:
                            wk_halves.append(load_w_half(*w_seq.pop(0)))
                        for ob in range(half * 8, half * 8 + 8):
                            oc = (ob % 8) * 128
                            hb = dr.tile([128, TH], bf16, name="hb")
                            for a, w in tok_tiles(tlen):
                                ps = pp.tile([128, 512], f32, name="pp")
                                mm3(ps[:, :w],
                                    [(w8[kp][:, :, oc:oc + 128],
                                      wlo[kp][:, :, oc:oc + 128])
                                     for kp in range(KP)],
                                    [(h8t[kp][:, :, t0 + a:t0 + a + w],
                                      hlot[kp][:, :, t0 + a:t0 + a + w])
                                     for kp in range(KP)])
                                nc.scalar.mul(hb[:, a:a + w], ps[:, :w], DESC)
                            # RoPE: rot-half via partition-swap SBUF->SBUF DMA
                            rot = dr.tile([128, TH], bf16, name="rot")
                            nc.sync.dma_start(rot[0:64], hb[64:128])
                            nc.sync.dma_start(rot[64:128], hb[0:64])
                            tmp = dr.tile([128, TH], bf16, name="tmp")
                            ro = dr.tile([128, TH], bf16, name="ro")
                            csl = slice(0, TH) if which == "K" else slice(WIN, TH)
                            n = TH if which == "K" else T
                            nc.vector.tensor_mul(tmp[:, :n], rot[:, :n],
                                                 sinb[:, csl])
                            nc.vector.tensor_mul(ro[:, :n], hb[:, :n],
                                                 cosb[:, csl])
                            nc.vector.tensor_add(ro[:, :n], ro[:, :n],
                                                 tmp[:, :n])
                            nc.sync.dma_start(DST[ob][:, :], ro[:, :n])

                # V natural: lhsT = h pair [128,2,128tok], rhs = Wv [128,2,512]
                for half in range(2):
                    wv8, wvlo = wk_halves.pop(0)
                    if w_seq:
                        wk_halves.append(load_w_half(*w_seq.pop(0)))
                    for tb in range(TH // 128):
                        tsl = slice(tb * 128, tb * 128 + 128)
                        vsb = vst.tile([128, DIMS // 2], bf16, name="vsb")
                        for og in range(2):
                            ps = pp.tile([128, 512], f32, name="pp")
                            ogs = slice(og * 512, (og + 1) * 512)
                            mm3(ps[:],
                                [(h8t[kp][:, :, tsl], hlot[kp][:, :, tsl])
                                 for kp in range(KP)],
                                [(wv8[kp][:, :, ogs], wvlo[kp][:, :, ogs])
                                 for kp in range(KP)])
                            nc.scalar.mul(vsb[:, ogs], ps[:], DESC)
                        nc.sync.dma_start(
                            VS[tb * 128:(tb + 1) * 128,
                               half * 1024:(half + 1) * 1024], vsb[:])

            # ---------------- P2+P3 fused per chunk ----------------
            if 2 in phases:
              with tc.tile_pool(name="wo", bufs=1) as wop, \
                 tc.tile_pool(name="pb", bufs=8) as pbp, \
                 tc.tile_pool(name="ot", bufs=12) as otp, \
                 tc.tile_pool(name="rb", bufs=3) as rbp, \
                 tc.tile_pool(name="ou", bufs=2) as oup, \
                 tc.tile_pool(name="ps_s", bufs=2, space="PSUM") as ps_s, \
                 tc.tile_pool(name="ps_d", bufs=2, space="PSUM") as ps_d, \
                 tc.tile_pool(name="ps_b", bufs=2, space="PSUM") as ps_b:
                # chunk-input loads go through the Pool engine's SWDGE queue:
                # the SP queue is clogged by P1's in-order stores, while Pool
                # is idle, so these prefetches fire as soon as data deps allow
                def load_kt(c0):
                    t = kq.tile([128, HEADS, WIN], bf16, name="kt")
                    nc.gpsimd.dma_start(t[:], KTS[:, :, c0:c0 + WIN]
                                        .rearrange("h d w -> d h w"))
                    return t

                kt_prev = load_kt(0)
                v_prev = kq.tile([128, 2, DIMS], bf16, name="v")
                nc.gpsimd.dma_start(
                    v_prev[:], VS[0:WIN].rearrange("(tb p) c -> p tb c", p=128))
                wo8 = wop.tile([128, HEADS, DIMS], f8, name="wo8")
                wolo = wop.tile([128, HEADS, DIMS], f8, name="wolo")
                for src, dst in ((WO8, wo8), (WOLO, wolo)):
                    wor = src.rearrange("(h p) d -> p h d", p=128)
                    for hg in range(4):
                        nc.gpsimd.dma_start(dst[:, hg * 4:(hg + 1) * 4],
                                            wor[:, hg * 4:(hg + 1) * 4])

                def front(c, h0, kts, qt):
                    """scores + exp + mask for one head pair."""
                    pbs2 = []
                    for h in (h0, h0 + 1):
                        # scores in one 2-bank psum: cols 0:512 = kb0|kb1,
                        # 512:768 = kb2, 768:896 = kb3 (q 128:256)
                        ps0 = ps_s.tile([128, 1024], f32, name="ps0")
                        for kb in range(2):
                            nc.tensor.matmul(
                                ps0[:, kb * WIN:(kb + 1) * WIN],
                                kts[kb][:, h, kb * 128:kb * 128 + 128],
                                qt[:, h], start=True, stop=True)
                        nc.tensor.matmul(
                            ps0[:, 512:512 + WIN], kts[2][:, h, 0:128],
                            qt[:, h], start=True, stop=True)
                        nc.tensor.matmul(
                            ps0[:, 768:896], kts[3][:, h, 128:256],
                            qt[:, h, 128:256], start=True, stop=True)
                        eb = pbp.tile([128, 896], bf16, name="eb")
                        if c == 0:
                            # pgate must only hit the prev-chunk columns
                            nc.scalar.activation(eb[:, 0:512], ps0[:, 0:512],
                                                 AF.Exp, bias=pgate[:],
                                                 scale=ISQ)
                            nc.scalar.activation(eb[:, 512:896],
                                                 ps0[:, 512:896], AF.Exp,
                                                 scale=ISQ)
                        else:
                            nc.scalar.activation(eb[:], ps0[:, 0:896], AF.Exp,
                                                 scale=ISQ)
                        pb1 = pbp.tile([128, 384], bf16, name="pb1")
                        nc.vector.tensor_mul(pb1[:], eb[:, 512:896], tri[:])
                        pbs2.append((eb, pb1))
                    return pbs2

                def back(pbs2, vs, ots):
                    """denominator + PV + normalize + fp8 split for a pair."""
                    pd = ps_d.tile([128, 512], f32, name="pd")
                    po = ps_b.tile([128, 512], f32, name="po")
                    for i in range(2):
                        pb0, pb1 = pbs2[i]
                        hc = i * WIN
                        # column-split PSUM groups: q 0:128 gets kb0-2,
                        # q 128:256 gets kb0-3
                        gA = [pb0[:, 0:128], pb0[:, WIN:WIN + 128],
                              pb1[:, 0:128]]
                        gB = [pb0[:, 128:WIN], pb0[:, WIN + 128:512],
                              pb1[:, 128:WIN], pb1[:, WIN:384]]
                        h = len(ots) * 2 + i
                        vA = [vs[0][:, 0, h * 128:(h + 1) * 128],
                              vs[1][:, 1, h * 128:(h + 1) * 128],
                              vs[2][:, 0, h * 128:(h + 1) * 128]]
                        vB = vA + [vs[3][:, 1, h * 128:(h + 1) * 128]]
                        for j, pbx in enumerate(gA):
                            nc.tensor.matmul(
                                pd[:, hc:hc + 128], onesm[:], pbx,
                                start=(j == 0), stop=(j == len(gA) - 1))
                        for j, pbx in enumerate(gB):
                            nc.tensor.matmul(
                                pd[:, hc + 128:hc + WIN], onesm[:], pbx,
                                start=(j == 0), stop=(j == len(gB) - 1))
                        for j, pbx in enumerate(gA):
                            nc.tensor.matmul(
                                po[:, hc:hc + 128], vA[j], pbx,
                                start=(j == 0), stop=(j == len(gA) - 1))
                        for j, pbx in enumerate(gB):
                            nc.tensor.matmul(
                                po[:, hc + 128:hc + WIN], vB[j], pbx,
                                start=(j == 0), stop=(j == len(gB) - 1))
                    rb = rbp.tile([128, 512], f32, name="rb")
                    with nc.allow_low_precision("softmax denominator"):
                        # onesm holds 0.125 so rb = 8/denominator and the
                        # normalized output lands at fp8 scale 8
                        nc.vector.reciprocal(rb[:], pd[:])
                    obf = otp.tile([128, 512], bf16, name="obf")
                    nc.vector.tensor_mul(obf[:], po[:], rb[:])
                    o8 = otp.tile([128, 512], f8, name="o8")
                    nc.scalar.copy(o8[:], obf[:])
                    olo = otp.tile([128, 512], f8, name="olo")
                    nc.vector.tensor_sub(olo[:], obf[:], o8[:])
                    ots.append((o8.rearrange("p (j q) -> p j q", j=2),
                                olo.rearrange("p (j q) -> p j q", j=2)))

                def emit_p3(c, ots):
                    # P3: out[tt] = sum_h O_h @ Wo_h, 3-term fp8 DoubleRow
                    # pairing the two heads of each pair
                    for tt in range(2):
                        ob_ = oup.tile([128, DIMS], bf16, name="ob")
                        for nt in range(4):
                            ps3 = ps_b.tile([128, 512], f32, name="po")
                            for hp_ in range(HEADS // 2):
                                o8s, olos = ots[hp_]
                                tsl = slice(tt * 128, tt * 128 + 128)
                                wsl8 = wo8[:, 2 * hp_:2 * hp_ + 2,
                                           nt * 512:(nt + 1) * 512]
                                wsll = wolo[:, 2 * hp_:2 * hp_ + 2,
                                            nt * 512:(nt + 1) * 512]
                                nc.tensor.matmul(
                                    ps3[:], o8s[:, :, tsl], wsl8,
                                    start=(hp_ == 0), stop=False, perf_mode=DR)
                                nc.tensor.matmul(
                                    ps3[:], o8s[:, :, tsl], wsll,
                                    start=False, stop=False, perf_mode=DR)
                                nc.tensor.matmul(
                                    ps3[:], olos[:, :, tsl], wsl8,
                                    start=False, stop=(hp_ == HEADS // 2 - 1),
                                    perf_mode=DR)
                            nc.scalar.mul(ob_[:, nt * 512:(nt + 1) * 512],
                                          ps3[:], DESC)
                        nc.sync.dma_start(
                            OUT[c * WIN + tt * 128:c * WIN + (tt + 1) * 128, :],
                            ob_[:])

                # software pipeline: pair i's denominator/PV (back) is emitted
                # two fronts later so the PE never waits on exp/mask or
                # semaphore props; each chunk's P3 slides similarly.
                from collections import deque
                backlog = deque()   # (pbs2, chunk, ots)
                vs_by_chunk = {}
                pend_p3 = None
                DEPTH = 3

                def run_back():
                    pbs2_, c_, ots_ = backlog.popleft()
                    back(pbs2_, vs_by_chunk[c_], ots_)

                for c in range(NC_):
                    kt_cur = load_kt(WIN + c * WIN)
                    qt = kq.tile([128, HEADS, WIN], bf16, name="qt")
                    nc.gpsimd.dma_start(qt[:], QTS[:, :, c * WIN:(c + 1) * WIN]
                                        .rearrange("h d w -> d h w"))

                    kts = [kt_prev, kt_prev, kt_cur, kt_cur]
                    ots = []
                    for h0 in range(0, HEADS, 2):
                        pbs2 = front(c, h0, kts, qt)
                        if len(backlog) >= DEPTH:
                            run_back()
                        if pend_p3 is not None and h0 == 6:
                            emit_p3(*pend_p3)
                            pend_p3 = None
                        if h0 == 4:
                            # v load sits after the last read of the buffer it
                            # rotates onto (the c-1 back just emitted above)
                            v_cur = kq.tile([128, 2, DIMS], bf16, name="v")
                            nc.gpsimd.dma_start(
                                v_cur[:],
                                VS[WIN + c * WIN:WIN + (c + 1) * WIN]
                                .rearrange("(tb p) c -> p tb c", p=128))
                            vs_by_chunk[c] = [v_prev, v_prev, v_cur, v_cur]
                        backlog.append((pbs2, c, ots))
                    pend_p3 = (c, ots)
                    kt_prev, v_prev = kt_cur, v_cur
                # flush the pipeline tail
                while backlog:
                    run_back()
                if pend_p3 is not None:
                    emit_p3(*pend_p3)
    return nc


def _q8(x, s):
    """Quantize x*s to fp8 e4m3; returns (fp8_array, residual_fp8_array)."""
    hi = (x * s).astype(ml_dtypes.float8_e4m3)
    lo = (x * s - hi.astype(np.float32)).astype(ml_dtypes.float8_e4m3)
    return hi, lo


def _host_inputs(hidden_states, Wq, Wk, Wv, Wo, T):
    """Build the 8 per-core input maps."""
    TH = T + WIN
    inv_freq = 1.0 / (THETA ** (np.arange(0, HD, 2, dtype=np.float32) / HD))

    qq = np.arange(WIN)[None, :]
    kk = np.arange(128)[:, None]
    tri = np.concatenate([(qq >= kk),
                          (qq[:, :128] >= kk)], 1).astype(ml_dtypes.bfloat16)
    # 0.125 so the softmax reciprocal bakes in the fp8 output scale of 8
    onesm_bf = np.full((128, 128), 0.125, ml_dtypes.bfloat16)

    # weights: fp8 hi/lo pairs in DoubleRow layout [KP*128, 2, DIMS]
    wts = {}
    for name, W in (("Q", Wq), ("K", Wk), ("V", Wv)):
        W = np.ascontiguousarray(W, np.float32)
        hi, lo = _q8(W, SH_W)
        # row r = (kp*2 + j)*128 + p  ->  layout [kp, p, j, c] -> [(kp p), j, c]
        hi = np.ascontiguousarray(
            hi.reshape(KP, 2, 128, DIMS).transpose(0, 2, 1, 3)
            .reshape(KP * 128, 2, DIMS))
        lo = np.ascontiguousarray(
            lo.reshape(KP, 2, 128, DIMS).transpose(0, 2, 1, 3)
            .reshape(KP * 128, 2, DIMS))
        wts[f"W{name}8"] = hi
        wts[f"W{name}LO"] = lo
    wo8, wolo = _q8(np.ascontiguousarray(Wo, np.float32), SH_W)

    in_maps = []
    for core in range(8):
        b, sh = divmod(core, NSH)
        t0 = sh * T
        hs = np.zeros((TH, DIMS), np.float32)
        lo_t = max(0, t0 - WIN)
        hs[WIN - (t0 - lo_t):] = hidden_states[b, lo_t:t0 + T]
        hT = np.ascontiguousarray(hs.T)
        h8, hlo = _q8(hT, SH_H)

        pos = np.arange(t0 - WIN, t0 + T, dtype=np.float32)
        f = np.outer(inv_freq, pos)                      # [64, TH]
        cos = np.concatenate([np.cos(f), np.cos(f)], 0)  # [128, TH]
        sin = np.sin(f)
        sins = np.concatenate([-sin, sin], 0)
        pg = np.full((128, 1), -1e30 if sh == 0 else 0.0, np.float32)
        in_maps.append({
            "H8": h8, "HLO": hlo, **wts, "WO8": wo8, "WOLO": wolo,
            "COS": cos.astype(ml_dtypes.bfloat16),
            "SINS": sins.astype(ml_dtypes.bfloat16),
            "TRI": tri, "PGATE": pg, "ONESM": onesm_bf,
        })
    return in_maps


_CACHE = {}


def run(hidden_states, Wq, Wk, Wv, Wo, T=S // NSH, **spmd_kwargs):
    key = T
    if key not in _CACHE:
        nc = bacc.Bacc(None)
        build(nc, T)
        nc.finalize()
        _CACHE[key] = nc
    nc = _CACHE[key]
    in_maps = _host_inputs(hidden_states, Wq, Wk, Wv, Wo, T)
    res = run_bass_kernel_spmd(nc, in_maps, core_ids=list(range(8)), **spmd_kwargs)
    outs = [res.results[i]["OUT"] for i in range(8)]
    full = np.empty((B, NSH * T, DIMS), np.float32)
    for core in range(8):
        b, sh = divmod(core, NSH)
        full[b, sh * T:(sh + 1) * T] = outs[core].astype(np.float32)
    return full, res


def kernel(hidden_states, Wq, Wk, Wv, Wo):
    out, _ = run(np.asarray(hidden_states), Wq, Wk, Wv, Wo)
    return out


# revision 43
# speedup vs baseline: 1.0119x; 1.0119x over previous
"""Block sliding-window attention on 8 TRN2 NeuronCores.

Sharding: sequence-parallel. 8 shards = (batch b in {0,1}) x (quarter s in
0..3); each core owns 2048 consecutive tokens of one batch plus a 256-token
K/V halo from the previous quarter (zeros + -inf gate for the first quarter).
No collectives: each core computes its tokens' full output rows.

v2 pipeline (vs v1: fp8 DoubleRow projections, RoPE fused into P1,
P2/P3 fused per chunk, no OTS round-trip):
  P1: Q^T/K^T/V via 3-term fp8 e4m3 hi/lo matmuls in DoubleRow perf mode
      (h = h8 + hlo at scale 8, W = W8 + Wlo at scale 256; terms
      h8*W8 + h8*Wlo + hlo*W8 share one PSUM at scale 2048; the dropped
      hlo*Wlo term is ~1e-3 relative). RoPE is applied to Q/K right after
      the PSUM drain (rot-half via SBUF->SBUF partition-swap DMA, cos/sin
      resident bf16), roped heads stored to DRAM scratch. V drains to DRAM
      natural-layout scratch.
  P2+P3 fused per 256-token chunk: scores S^T = K Q^T per 128-key block
      (skipping the fully-masked kb3 x first-half-queries block), exp on
      ACT (-1e30 bias gates the no-previous case), 0/1 triangular mask on
      DVE, denominator via all-ones matmul, O^T = V^T P^T, normalize
      (reciprocal on DVE, multiply on Pool engine), then immediately
      out += O_h @ Wo_h accumulated over 16 heads in PSUM - O^T never
      leaves SBUF.
"""
import sys

try:
    import concourse  # noqa: F401
except ImportError:
    sys.path.insert(0, '/opt/trn_rl_repo')

import ml_dtypes
import numpy as np

import concourse.bacc as bacc
import concourse.mybir as mybir
import concourse.tile as tile
from concourse.bass_utils import run_bass_kernel_spmd

f32 = mybir.dt.float32
f32r = mybir.dt.float32r
bf16 = mybir.dt.bfloat16
f8 = mybir.dt.float8e4
AF = mybir.ActivationFunctionType
DR = mybir.MatmulPerfMode.DoubleRow

DIMS = 2048
HEADS = 16
HD = 128           # head dim
WIN = 256          # window / chunk
B, S = 2, 8192
NSH = 4            # seq shards per batch
THETA = 10000.0
ISQ = float(1.0 / np.sqrt(HD))
KP = DIMS // 256   # 8 contraction k-pairs (256 rows each) for DoubleRow
SH_H = 8.0         # fp8 scale for hidden
SH_W = 256.0       # fp8 scale for weights
DESC = 1.0 / (SH_H * SH_W)


def tok_tiles(n, w=512):
    out, a = [], 0
    while a < n:
        out.append((a, min(w, n - a)))
        a += w
    return out


def build(nc, T, phases=(1, 2)):
    """Emit the per-core program. T = local tokens (multiple of 512)."""
    TH = T + WIN                      # with halo
    NC_ = T // WIN                    # chunks
    H8 = nc.dram_tensor("H8", [DIMS, TH], f8, kind="ExternalInput")
    HLO = nc.dram_tensor("HLO", [DIMS, TH], f8, kind="ExternalInput")
    WT8 = {}
    WTLO = {}
    for w_ in ("Q", "K", "V"):
        WT8[w_] = nc.dram_tensor(f"W{w_}8", [KP * 128, 2, DIMS], f8,
                                 kind="ExternalInput")
        WTLO[w_] = nc.dram_tensor(f"W{w_}LO", [KP * 128, 2, DIMS], f8,
                                  kind="ExternalInput")
    WO8 = nc.dram_tensor("WO8", [HEADS * 128, DIMS], f8, kind="ExternalInput")
    WOLO = nc.dram_tensor("WOLO", [HEADS * 128, DIMS], f8, kind="ExternalInput")
    COS = nc.dram_tensor("COS", [HD, TH], bf16, kind="ExternalInput")
    SINS = nc.dram_tensor("SINS", [HD, TH], bf16, kind="ExternalInput")
    TRI = nc.dram_tensor("TRI", [128, 384], bf16, kind="ExternalInput")
    PGATE = nc.dram_tensor("PGATE", [128, 1], f32, kind="ExternalInput")
    ONESM = nc.dram_tensor("ONESM", [128, 128], bf16, kind="ExternalInput")
    OUT = nc.dram_tensor("OUT", [T, DIMS], bf16, kind="ExternalOutput")

    KTS = nc.dram_tensor("KTS", [HEADS, HD, TH], bf16)   # roped K^T
    QTS = nc.dram_tensor("QTS", [HEADS, HD, T], bf16)    # roped Q^T
    VS = nc.dram_tensor("VS", [TH, DIMS], bf16)          # V natural

    with tile.TileContext(nc) as tc:
        # kq is opened at the outermost scope: its space never overlaps the
        # P1 pools, so P23 chunk-input prefetches have no buffer-reuse (WAR)
        # dependency on P1's tail and can land as soon as KTS/QTS/VS rows are
        # written.
        with tc.tile_pool(name="cst", bufs=1) as cst, \
             tc.tile_pool(name="kq", bufs=2) as kq:
            tri = cst.tile([128, 384], bf16)
            pgate = cst.tile([128, 1], f32)
            onesm = cst.tile([128, 128], bf16)
            nc.sync.dma_start(tri[:], TRI[:])
            nc.sync.dma_start(pgate[:], PGATE[:])
            nc.sync.dma_start(onesm[:], ONESM[:])

            # ---------------- P1: projections + RoPE ----------------
            if 1 in phases:
              with tc.tile_pool(name="hp", bufs=1) as hp, \
                 tc.tile_pool(name="rope_c", bufs=1) as rcp, \
                 tc.tile_pool(name="wp", bufs=1) as wp, \
                 tc.tile_pool(name="dr", bufs=2) as dr, \
                 tc.tile_pool(name="vst", bufs=2) as vst, \
                 tc.tile_pool(name="pp", bufs=8, space="PSUM") as pp:
                def load_w_half(which, half):
                    """Load output-column half of a weight pair."""
                    w8, wlo = [], []
                    csl = slice(half * DIMS // 2, (half + 1) * DIMS // 2)
                    for kp in range(KP):
                        a = wp.tile([128, 2, DIMS // 2], f8, name=f"w8_{kp}")
                        b_ = wp.tile([128, 2, DIMS // 2], f8, name=f"wlo_{kp}")
                        nc.sync.dma_start(a[:], WT8[which]
                                          .rearrange("(kp p) j c -> kp p j c",
                                                     p=128)[kp][:, :, csl])
                        nc.sync.dma_start(b_[:], WTLO[which]
                                          .rearrange("(kp p) j c -> kp p j c",
                                                     p=128)[kp][:, :, csl])
                        w8.append(a)
                        wlo.append(b_)
                    return w8, wlo

                # startup: interleave K-weight half 0 with the hidden loads
                # kp-wise so the kp0 matmuls can begin after ~2.4MB of DMA
                wk0_8, wk0_lo = [], []
                h8t, hlot = [], []
                for kp in range(KP):
                    wa = wp.tile([128, 2, DIMS // 2], f8, name=f"w8_{kp}")
                    wb = wp.tile([128, 2, DIMS // 2], f8, name=f"wlo_{kp}")
                    csl = slice(0, DIMS // 2)
                    nc.sync.dma_start(wa[:], WT8["K"]
                                      .rearrange("(kp p) j c -> kp p j c",
                                                 p=128)[kp][:, :, csl])
                    nc.sync.dma_start(wb[:], WTLO["K"]
                                      .rearrange("(kp p) j c -> kp p j c",
                                                 p=128)[kp][:, :, csl])
                    wk0_8.append(wa)
                    wk0_lo.append(wb)
                    a = hp.tile([128, 2, TH], f8, name=f"h8_{kp}")
                    b_ = hp.tile([128, 2, TH], f8, name=f"hlo_{kp}")
                    sl = slice(kp * 256, (kp + 1) * 256)
                    nc.sync.dma_start(
                        a[:], H8[sl].rearrange("(j p) t -> p j t", p=128))
                    nc.sync.dma_start(
                        b_[:], HLO[sl].rearrange("(j p) t -> p j t", p=128))
                    h8t.append(a)
                    hlot.append(b_)
                wk_halves = [(wk0_8, wk0_lo)]
                cosb = rcp.tile([128, TH], bf16)
                sinb = rcp.tile([128, TH], bf16)
                nc.sync.dma_start(cosb[:], COS[:])
                nc.sync.dma_start(sinb[:], SINS[:])

                def mm3(ps, lhs, rhs):
                    """3-term fp8 DoubleRow accumulate into one psum."""
                    n = len(lhs)
                    for kp in range(n):
                        (l8, llo), (r8, rlo) = lhs[kp], rhs[kp]
                        nc.tensor.matmul(ps, l8, r8, start=(kp == 0),
                                         stop=False, perf_mode=DR)
                        nc.tensor.matmul(ps, l8, rlo, start=False, stop=False,
                                         perf_mode=DR)
                        nc.tensor.matmul(ps, llo, r8, start=False,
                                         stop=(kp == n - 1), perf_mode=DR)

                # K then Q (head-transposed layout + RoPE), V natural last.
                # Weights stream in output-column halves.
                w_seq = [("K", 1), ("Q", 0), ("Q", 1), ("V", 0), ("V", 1)]
                for which, DST, t0, tlen in (("K", KTS, 0, TH),
                                             ("Q", QTS, WIN, T)):
                    for half in range(2):
                        w8, wlo = wk_halves.pop(0)
                        if w_seq:
                            wk_halves.append(load_w_half(*w_seq.pop(0)))
                        for ob in range(half * 8, half * 8 + 8):
                            oc = (ob % 8) * 128
                            hb = dr.tile([128, TH], bf16, name="hb")
                            for a, w in tok_tiles(tlen):
                                ps = pp.tile([128, 512], f32, name="pp")
                                mm3(ps[:, :w],
                                    [(w8[kp][:, :, oc:oc + 128],
                                      wlo[kp][:, :, oc:oc + 128])
                                     for kp in range(KP)],
                                    [(h8t[kp][:, :, t0 + a:t0 + a + w],
                                      hlot[kp][:, :, t0 + a:t0 + a + w])
                                     for kp in range(KP)])
                                nc.scalar.mul(hb[:, a:a + w], ps[:, :w], DESC)
                            # RoPE: rot-half via partition-swap SBUF->SBUF DMA
                            rot = dr.tile([128, TH], bf16, name="rot")
                            nc.sync.dma_start(rot[0:64], hb[64:128])
                            nc.sync.dma_start(rot[64:128], hb[0:64])
                            tmp = dr.tile([128, TH], bf16, name="tmp")
                            ro = dr.tile([128, TH], bf16, name="ro")
                            csl = slice(0, TH) if which == "K" else slice(WIN, TH)
                            n = TH if which == "K" else T
                            nc.vector.tensor_mul(tmp[:, :n], rot[:, :n],
                                                 sinb[:, csl])
                            nc.vector.tensor_mul(ro[:, :n], hb[:, :n],
                                                 cosb[:, csl])
                            nc.vector.tensor_add(ro[:, :n], ro[:, :n],
                                                 tmp[:, :n])
                            nc.sync.dma_start(DST[ob][:, :], ro[:, :n])

                # V natural: lhsT = h pair [128,2,128tok], rhs = Wv [128,2,512]
                for half in range(2):
                    wv8, wvlo = wk_halves.pop(0)
                    if w_seq:
                        wk_halves.append(load_w_half(*w_seq.pop(0)))
                    for tb in range(TH // 128):
                        tsl = slice(tb * 128, tb * 128 + 128)
                        vsb = vst.tile([128, DIMS // 2], bf16, name="vsb")
                        for og in range(2):
                            ps = pp.tile([128, 512], f32, name="pp")
                            ogs = slice(og * 512, (og + 1) * 512)
                            mm3(ps[:],
                                [(h8t[kp][:, :, tsl], hlot[kp][:, :, tsl])
                                 for kp in range(KP)],
                                [(wv8[kp][:, :, ogs], wvlo[kp][:, :, ogs])
                                 for kp in range(KP)])
                            nc.scalar.mul(vsb[:, ogs], ps[:], DESC)
                        nc.sync.dma_start(
                            VS[tb * 128:(tb + 1) * 128,
                               half * 1024:(half + 1) * 1024], vsb[:])

            # ---------------- P2+P3 fused per chunk ----------------
            if 2 in phases:
              with tc.tile_pool(name="wo", bufs=1) as wop, \
                 tc.tile_pool(name="pb", bufs=8) as pbp, \
                 tc.tile_pool(name="ot", bufs=12) as otp, \
                 tc.tile_pool(name="rb", bufs=3) as rbp, \
                 tc.tile_pool(name="ou", bufs=2) as oup, \
                 tc.tile_pool(name="ps_s", bufs=2, space="PSUM") as ps_s, \
                 tc.tile_pool(name="ps_d", bufs=2, space="PSUM") as ps_d, \
                 tc.tile_pool(name="ps_b", bufs=2, space="PSUM") as ps_b:
                # chunk-input loads go through the Pool engine's SWDGE queue:
                # the SP queue is clogged by P1's in-order stores, while Pool
                # is idle, so these prefetches fire as soon as data deps allow
                def load_kt(c0):
                    t = kq.tile([128, HEADS, WIN], bf16, name="kt")
                    nc.gpsimd.dma_start(t[:], KTS[:, :, c0:c0 + WIN]
                                        .rearrange("h d w -> d h w"))
                    return t

                kt_prev = load_kt(0)
                v_prev = kq.tile([128, 2, DIMS], bf16, name="v")
                nc.gpsimd.dma_start(
                    v_prev[:], VS[0:WIN].rearrange("(tb p) c -> p tb c", p=128))
                wo8 = wop.tile([128, HEADS, DIMS], f8, name="wo8")
                wolo = wop.tile([128, HEADS, DIMS], f8, name="wolo")
                for src, dst in ((WO8, wo8), (WOLO, wolo)):
                    wor = src.rearrange("(h p) d -> p h d", p=128)
                    for hg in range(4):
                        nc.gpsimd.dma_start(dst[:, hg * 4:(hg + 1) * 4],
                                            wor[:, hg * 4:(hg + 1) * 4])

                def front(c, h0, kts, qt):
                    """scores + exp + mask for one head pair."""
                    pbs2 = []
                    for h in (h0, h0 + 1):
                        # scores: pb0 = P(kb0|kb1) [128,512],
                        # pb1 = P(kb2 q0:256 | kb3 q128:256) [128,384]
                        ps0 = ps_s.tile([128, 512], f32, name="ps0")
                        for kb in range(2):
                            nc.tensor.matmul(
                                ps0[:, kb * WIN:(kb + 1) * WIN],
                                kts[kb][:, h, kb * 128:kb * 128 + 128],
                                qt[:, h], start=True, stop=True)
                        ps1 = ps_s.tile([128, 512], f32, name="ps1")
                        nc.tensor.matmul(
                            ps1[:, 0:WIN], kts[2][:, h, 0:128],
                            qt[:, h], start=True, stop=True)
                        nc.tensor.matmul(
                            ps1[:, WIN:WIN + 128], kts[3][:, h, 128:256],
                            qt[:, h, 128:256], start=True, stop=True)
                        pb0 = pbp.tile([128, 512], bf16, name="pb0")
                        if c == 0:
                            nc.scalar.activation(pb0[:], ps0[:], AF.Exp,
                                                 bias=pgate[:], scale=ISQ)
                        else:
                            nc.scalar.activation(pb0[:], ps0[:], AF.Exp,
                                                 scale=ISQ)
                        eb1 = pbp.tile([128, 384], bf16, name="eb1")
                        nc.scalar.activation(eb1[:], ps1[:, 0:384], AF.Exp,
                                             scale=ISQ)
                        pb1 = pbp.tile([128, 384], bf16, name="pb1")
                        nc.vector.tensor_mul(pb1[:], eb1[:], tri[:])
                        pbs2.append((pb0, pb1))
                    return pbs2

                def back(pbs2, vs, ots):
                    """denominator + PV + normalize + fp8 split for a pair."""
                    pd = ps_d.tile([128, 512], f32, name="pd")
                    po = ps_b.tile([128, 512], f32, name="po")
                    for i in range(2):
                        pb0, pb1 = pbs2[i]
                        hc = i * WIN
                        # column-split PSUM groups: q 0:128 gets kb0-2,
                        # q 128:256 gets kb0-3
                        gA = [pb0[:, 0:128], pb0[:, WIN:WIN + 128],
                              pb1[:, 0:128]]
                        gB = [pb0[:, 128:WIN], pb0[:, WIN + 128:512],
                              pb1[:, 128:WIN], pb1[:, WIN:384]]
                        h = len(ots) * 2 + i
                        vA = [vs[0][:, 0, h * 128:(h + 1) * 128],
                              vs[1][:, 1, h * 128:(h + 1) * 128],
                              vs[2][:, 0, h * 128:(h + 1) * 128]]
                        vB = vA + [vs[3][:, 1, h * 128:(h + 1) * 128]]
                        for j, pbx in enumerate(gA):
                            nc.tensor.matmul(
                                pd[:, hc:hc + 128], onesm[:], pbx,
                                start=(j == 0), stop=(j == len(gA) - 1))
                        for j, pbx in enumerate(gB):
                            nc.tensor.matmul(
                                pd[:, hc + 128:hc + WIN], onesm[:], pbx,
                                start=(j == 0), stop=(j == len(gB) - 1))
                        for j, pbx in enumerate(gA):
                            nc.tensor.matmul(
                                po[:, hc:hc + 128], vA[j], pbx,
                                start=(j == 0), stop=(j == len(gA) - 1))
                        for j, pbx in enumerate(gB):
                            nc.tensor.matmul(
                                po[:, hc + 128:hc + WIN], vB[j], pbx,
                                start=(j == 0), stop=(j == len(gB) - 1))
                    rb = rbp.tile([128, 512], f32, name="rb")
                    with nc.allow_low_precision("softmax denominator"):
                        # onesm holds 0.125 so rb = 8/denominator and the
                        # normalized output lands at fp8 scale 8
                        nc.vector.reciprocal(rb[:], pd[:])
                    obf = otp.tile([128, 512], bf16, name="obf")
                    nc.vector.tensor_mul(obf[:], po[:], rb[:])
                    o8 = otp.tile([128, 512], f8, name="o8")
                    nc.scalar.copy(o8[:], obf[:])
                    olo = otp.tile([128, 512], f8, name="olo")
                    nc.vector.tensor_sub(olo[:], obf[:], o8[:])
                    ots.append((o8.rearrange("p (j q) -> p j q", j=2),
                                olo.rearrange("p (j q) -> p j q", j=2)))

                def emit_p3(c, ots):
                    # P3: out[tt] = sum_h O_h @ Wo_h, 3-term fp8 DoubleRow
                    # pairing the two heads of each pair
                    for tt in range(2):
                        ob_ = oup.tile([128, DIMS], bf16, name="ob")
                        for nt in range(4):
                            ps3 = ps_b.tile([128, 512], f32, name="po")
                            for hp_ in range(HEADS // 2):
                                o8s, olos = ots[hp_]
                                tsl = slice(tt * 128, tt * 128 + 128)
                                wsl8 = wo8[:, 2 * hp_:2 * hp_ + 2,
                                           nt * 512:(nt + 1) * 512]
                                wsll = wolo[:, 2 * hp_:2 * hp_ + 2,
                                            nt * 512:(nt + 1) * 512]
                                nc.tensor.matmul(
                                    ps3[:], o8s[:, :, tsl], wsl8,
                                    start=(hp_ == 0), stop=False, perf_mode=DR)
                                nc.tensor.matmul(
                                    ps3[:], o8s[:, :, tsl], wsll,
                                    start=False, stop=False, perf_mode=DR)
                                nc.tensor.matmul(
                                    ps3[:], olos[:, :, tsl], wsl8,
                                    start=False, stop=(hp_ == HEADS // 2 - 1),
                                    perf_mode=DR)
                            nc.scalar.mul(ob_[:, nt * 512:(nt + 1) * 512],
                                          ps3[:], DESC)
                        nc.sync.dma_start(
                            OUT[c * WIN + tt * 128:c * WIN + (tt + 1) * 128, :],
                            ob_[:])

                # software pipeline: pair i's denominator/PV (back) is emitted
                # two fronts later so the PE never waits on exp/mask or
                # semaphore props; each chunk's P3 slides similarly.
                from collections import deque
                backlog = deque()   # (pbs2, chunk, ots)
                vs_by_chunk = {}
                pend_p3 = None
                DEPTH = 3

                def run_back():
                    pbs2_, c_, ots_ = backlog.popleft()
                    back(pbs2_, vs_by_chunk[c_], ots_)

                for c in range(NC_):
                    kt_cur = load_kt(WIN + c * WIN)
                    qt = kq.tile([128, HEADS, WIN], bf16, name="qt")
                    nc.gpsimd.dma_start(qt[:], QTS[:, :, c * WIN:(c + 1) * WIN]
                                        .rearrange("h d w -> d h w"))

                    kts = [kt_prev, kt_prev, kt_cur, kt_cur]
                    ots = []
                    for h0 in range(0, HEADS, 2):
                        pbs2 = front(c, h0, kts, qt)
                        if len(backlog) >= DEPTH:
                            run_back()
                        if pend_p3 is not None and h0 == 6:
                            emit_p3(*pend_p3)
                            pend_p3 = None
                        if h0 == 4:
                            # v load sits after the last read of the buffer it
                            # rotates onto (the c-1 back just emitted above)
                            v_cur = kq.tile([128, 2, DIMS], bf16, name="v")
                            nc.gpsimd.dma_start(
                                v_cur[:],
                                VS[WIN + c * WIN:WIN + (c + 1) * WIN]
                                .rearrange("(tb p) c -> p tb c", p=128))
                            vs_by_chunk[c] = [v_prev, v_prev, v_cur, v_cur]
                        backlog.append((pbs2, c, ots))
                    pend_p3 = (c, ots)
                    kt_prev, v_prev = kt_cur, v_cur
                # flush the pipeline tail
                while backlog:
                    run_back()
                if pend_p3 is not None:
                    emit_p3(*pend_p3)
    return nc


def _q8(x, s):
    """Quantize x*s to fp8 e4m3; returns (fp8_array, residual_fp8_array)."""
    hi = (x * s).astype(ml_dtypes.float8_e4m3)
    lo = (x * s - hi.astype(np.float32)).astype(ml_dtypes.float8_e4m3)
    return hi, lo


def _host_inputs(hidden_states, Wq, Wk, Wv, Wo, T):
    """Build the 8 per-core input maps."""
    TH = T + WIN
    inv_freq = 1.0 / (THETA ** (np.arange(0, HD, 2, dtype=np.float32) / HD))

    qq = np.arange(WIN)[None, :]
    kk = np.arange(128)[:, None]
    tri = np.concatenate([(qq >= kk),
                          (qq[:, :128] >= kk)], 1).astype(ml_dtypes.bfloat16)
    # 0.125 so the softmax reciprocal bakes in the fp8 output scale of 8
    onesm_bf = np.full((128, 128), 0.125, ml_dtypes.bfloat16)

    # weights: fp8 hi/lo pairs in DoubleRow layout [KP*128, 2, DIMS]
    wts = {}
    for name, W in (("Q", Wq), ("K", Wk), ("V", Wv)):
        W = np.ascontiguousarray(W, np.float32)
        hi, lo = _q8(W, SH_W)
        # row r = (kp*2 + j)*128 + p  ->  layout [kp, p, j, c] -> [(kp p), j, c]
        hi = np.ascontiguousarray(
            hi.reshape(KP, 2, 128, DIMS).transpose(0, 2, 1, 3)
            .reshape(KP * 128, 2, DIMS))
        lo = np.ascontiguousarray(
            lo.reshape(KP, 2, 128, DIMS).transpose(0, 2, 1, 3)
            .reshape(KP * 128, 2, DIMS))
        wts[f"W{name}8"] = hi
        wts[f"W{name}LO"] = lo
    wo8, wolo = _q8(np.ascontiguousarray(Wo, np.float32), SH_W)

    in_maps = []
    for core in range(8):
        b, sh = divmod(core, NSH)
        t0 = sh * T
        hs = np.zeros((TH, DIMS), np.float32)
        lo_t = max(0, t0 - WIN)
        hs[WIN - (t0 - lo_t):] = hidden_states[b, lo_t:t0 + T]
        hT = np.ascontiguousarray(hs.T)
        h8, hlo = _q8(hT, SH_H)

        pos = np.arange(t0 - WIN, t0 + T, dtype=np.float32)
        f = np.outer(inv_freq, pos)                      # [64, TH]
        cos = np.concatenate([np.cos(f), np.cos(f)], 0)  # [128, TH]
        sin = np.sin(f)
        sins = np.concatenate([-sin, sin], 0)
        pg = np.full((128, 1), -1e30 if sh == 0 else 0.0, np.float32)
        in_maps.append({
            "H8": h8, "HLO": hlo, **wts, "WO8": wo8, "WOLO": wolo,
            "COS": cos.astype(ml_dtypes.bfloat16),
            "SINS": sins.astype(ml_dtypes.bfloat16),
            "TRI": tri, "PGATE": pg, "ONESM": onesm_bf,
        })
    return in_maps


_CACHE = {}


def run(hidden_states, Wq, Wk, Wv, Wo, T=S // NSH, **spmd_kwargs):
    key = T
    if key not in _CACHE:
        nc = bacc.Bacc(None)
        build(nc, T)
        nc.finalize()
        _CACHE[key] = nc
    nc = _CACHE[key]
    in_maps = _host_inputs(hidden_states, Wq, Wk, Wv, Wo, T)
    res = run_bass_kernel_spmd(nc, in_maps, core_ids=list(range(8)), **spmd_kwargs)
    outs = [res.results[i]["OUT"] for i in range(8)]
    full = np.empty((B, NSH * T, DIMS), np.float32)
    for core in range(8):
        b, sh = divmod(core, NSH)
        full[b, sh * T:(sh + 1) * T] = outs[core].astype(np.float32)
    return full, res


def kernel(hidden_states, Wq, Wk, Wv, Wo):
    out, _ = run(np.asarray(hidden_states), Wq, Wk, Wv, Wo)
    return out
